# revision 28
# baseline (speedup 1.0000x reference)
"""Trainium2 Bass kernel for MultiHeadAttention + residual + BatchNorm.

Model (reference):
  q = query @ Wq.T ; k = key @ Wk.T ; v = key @ Wv.T    (per-head split)
  score = q k^T / sqrt(D), causal mask, softmax over keys
  res   = (attn @ v) + query
  out   = batchnorm(res over all (N*L) rows, per feature) * gamma + beta

Sharding over 8 cores: core c -> (batch n = c % 4, head-block hb = c // 4).
Each core computes its batch's 8 heads (512 of the 1024 features).
BatchNorm statistics are all-gathered across the 4 cores sharing a head
block (replica groups [[0,1,2,3],[4,5,6,7]]).

Everything on-device stays in [feature, length] (transposed) layout:
  - activations stream in as q^T/k^T [D, l] bf16 (host pre-transposes and
    pre-casts; host also permutes the contraction dim so this core's own
    512 residual dims come first, with W rows permuted identically)
  - attention is computed per head as score^T [j, i] and out^T [d, i],
    so no PE transposes are ever needed
  - BN stats are free-dim reductions and BN apply uses per-partition
    scalars; output is written transposed [F, l] and the host transposes
    back.

Causal masking is trapezoidal: fully-masked (j>i) blocks are never
computed; diagonal jc blocks use narrowed matmuls/exps plus an
affine_select for the partial 128x128 block. Score pairs share one
2-bank PSUM tile so each exp instruction covers two j-blocks
(amortizes the Act access bubble).

DMAs are batched ([128, 4096] per chunk-side) because each DMA costs
~2.2us on the issuing sequencer regardless of size.
"""

import math
import sys

sys.path.insert(0, "/opt/trn_rl_repo")

import numpy as np

import concourse.bass as bass
import concourse.mybir as mybir
from concourse import bacc
import concourse.tile as tile
from concourse import bass_utils

F32 = mybir.dt.float32
BF16 = mybir.dt.bfloat16
F8 = mybir.dt.float8e4
DR = mybir.MatmulPerfMode.DoubleRow
AX = mybir.AxisListType.X
ALU = mybir.AluOpType

N = 4
L = 2048
D = 1024
H = 16
P = 64
NCORES = 8
NB = 4            # batches
HBS = 2           # head blocks
F = D // HBS      # features per core = 512
H8 = H // HBS     # heads per core = 8
EPS = 1e-5
SCALE = 1.0 / math.sqrt(D)

_cached = {}


def build_program(l=L):
    """Build the SPMD Bass program (identical on all 8 cores)."""
    ic_n = l // 512
    nrows_total = float(NB * l)

    nc = bacc.Bacc("TRN2", target_bir_lowering=False, debug=False,
                   num_devices=NCORES)

    qt_nd = nc.dram_tensor("qt_nd", [D, l], F8, kind="ExternalInput").ap()
    kt_nd = nc.dram_tensor("kt_nd", [D, l], F8, kind="ExternalInput").ap()
    qres_nd = nc.dram_tensor("qres_nd", [F, l], BF16,
                             kind="ExternalInput").ap()
    wqt = nc.dram_tensor("wqt", [128, 4096], F8, kind="ExternalInput").ap()
    wkt = nc.dram_tensor("wkt", [128, 4096], F8, kind="ExternalInput").ap()
    wvt = nc.dram_tensor("wvt", [128, 4096], F8, kind="ExternalInput").ap()
    gamma = nc.dram_tensor("gamma", [1, F], F32, kind="ExternalInput").ap()
    beta = nc.dram_tensor("beta", [1, F], F32, kind="ExternalInput").ap()
    out_s = nc.dram_tensor("out_s", [F, l], BF16, kind="ExternalOutput").ap()

    with tile.TileContext(nc) as tc, \
         tc.tile_pool(name="consts", bufs=1) as consts, \
         tc.tile_pool(name="persist", bufs=1) as persist, \
         tc.tile_pool(name="wt", bufs=1) as wtp, \
         tc.tile_pool(name="qxt", bufs=3) as qxtp, \
         tc.tile_pool(name="qrp", bufs=3) as qrp, \
         tc.tile_pool(name="kxt", bufs=2) as kxtp, \
         tc.tile_pool(name="qtp", bufs=2) as qtp, \
         tc.tile_pool(name="atp", bufs=3) as atp, \
         tc.tile_pool(name="rrp", bufs=3) as rrp, \
         tc.tile_pool(name="rbp", bufs=3) as rbp, \
         tc.tile_pool(name="sqp", bufs=2) as sqp, \
         tc.tile_pool(name="redp", bufs=4) as redp, \
         tc.tile_pool(name="bnp", bufs=1) as bnp, \
         tc.tile_pool(name="outp", bufs=4) as outp, \
         tc.tile_pool(name="pja", bufs=2, space="PSUM") as pja, \
         tc.tile_pool(name="stp", bufs=2, space="PSUM") as stpp, \
         tc.tile_pool(name="otp", bufs=2, space="PSUM") as otpp, \
         tc.tile_pool(name="dram", bufs=1, space="DRAM") as dramp:

        eps_sb = consts.tile([128, 1], F32)
        nc.vector.memset(eps_sb, EPS)

        kt_sb = persist.tile([128, 4 * l], BF16, tag="kt")
        v_sb = persist.tile([128, (l // 256) * 1280], F8, tag="v")
        res_sb = persist.tile([128, 4 * l], F32, tag="res")
        accs = [persist.tile([128, 8], F32, tag="acc0", name="acc0"),
                persist.tile([128, 8], F32, tag="acc1", name="acc1")]

        # layout: (jc-pair, head, jc-parity, 64 v dims + ones + 15 pad) --
        # pair-contiguous so the fp8 DoubleRow AV matmul loads both jc
        # blocks as one dual-weight pair; stationary width padded to 80
        # (dual-fp8 ldweights requires a multiple of 16)
        v_v = v_sb.rearrange("p (jp h t x) -> p jp h t x", h=H8, t=2, x=80)
        nc.gpsimd.memset(v_v[:, :, :, :, 64:65], 1.0)
        nc.gpsimd.memset(v_v[:, :, :, :, 65:80], 0.0)

        def dma_xt(ic, side, eng=None, split=False):
            """One batched DMA: all 8 dc-blocks of a 512-col chunk."""
            src = qt_nd if side == "q" else kt_nd
            pool = qxtp if side == "q" else kxtp
            xt = pool.tile([128, 4096], F8, tag="xt", name=f"{side}xt")
            xtv = xt.rearrange("p (dc c) -> p dc c", dc=8)
            halves = ((0, 4), (4, 8)) if split else ((0, 8),)
            for d0, d1 in halves:
                src_ap = bass.AP(
                    tensor=src.tensor,
                    offset=src.offset + ic * 512 + d0 * 128 * l,
                    ap=[[l, 128], [128 * l, d1 - d0], [1, 512]])
                (eng or nc.sync).dma_start(xtv[:, d0:d1], src_ap)
            return xt

        def dma_qres(ic):
            qr = qrp.tile([128, 2048], BF16, tag="qr", name="qr")
            src_ap = bass.AP(
                tensor=qres_nd.tensor, offset=qres_nd.offset + ic * 512,
                ap=[[l, 128], [128 * l, 4], [1, 512]])
            nc.sync.dma_start(qr.rearrange("p (dc c) -> p dc c", dc=4),
                              src_ap)
            return qr

        # prefetch, all on SP in need-order: wqt, q-chunk0 (split so the
        # first projection group can start after half), then the k side
        wts = {}
        for wname in ("wqt", "wkt", "wvt"):
            wts[wname] = wtp.tile([128, 4096], F8, tag=wname, name=wname)

        def dma_w(wname, wdram):
            nc.sync.dma_start(wts[wname], wdram)

        dma_w("wqt", wqt)
        qxt0 = dma_xt(0, "q", split=True)
        kxt0 = dma_xt(0, "k")
        dma_w("wkt", wkt)
        dma_w("wvt", wvt)
        qres0 = dma_qres(0)

        def emit_A_groups(ic, qxt, kxt, on_act=False):
            """12 closures: 4 q-proj, 4 k-proj, 4 v-proj groups.

            on_act: issue the PSUM->SBUF copies on the Activation engine
            (it is starved during the early, PE-bound chunks; DVE keeps
            the late-chunk copies when Act is the bottleneck)."""
            qt_ic = qtp.tile([128, 4 * 512], BF16, tag="qt", name="qt_ic")
            copy = nc.scalar.copy if on_act else nc.vector.tensor_copy
            groups = []

            def qk_group(side, oc):
                def go():
                    xt = qxt if side == "q" else kxt
                    wt_use = wts["wqt"] if side == "q" else wts["wkt"]
                    xv_ = xt.rearrange("p (dc c) -> p dc c", dc=8)
                    pj = pja.tile([128, 512], F32, tag="pj", name="pj")
                    for dp in range(4):
                        wsl = wt_use[:, dp * 1024 + oc * 256:
                                     dp * 1024 + oc * 256 + 256]
                        nc.tensor.matmul(
                            pj,
                            wsl.rearrange("p (two c) -> p two c", two=2),
                            xv_[:, 2 * dp:2 * dp + 2, :],
                            start=(dp == 0), stop=(dp == 3),
                            perf_mode=DR)
                    if side == "q":
                        copy(qt_ic[:, oc * 512:(oc + 1) * 512], pj)
                    else:
                        copy(kt_sb[:, oc * l + ic * 512:
                                   oc * l + ic * 512 + 512], pj)
                return go

            def v_group(lsub):
                def go():
                    pj = pja.tile([128, 512], F32, tag="pj", name="pj")
                    for dc in range(8):
                        nc.tensor.matmul(
                            pj,
                            kxt[:, dc * 512 + lsub * 128:
                                dc * 512 + lsub * 128 + 128],
                            wts["wvt"][:, dc * 512:(dc + 1) * 512],
                            start=(dc == 0), stop=(dc == 7))
                    jc = ic * 4 + lsub
                    jp, tpar = jc // 2, jc % 2
                    vdst = v_v[:, jp, :, tpar, 0:64]
                    vsrc = pj.rearrange("p (h x) -> p h x", h=8)
                    copy(vdst, vsrc)
                return go

            for oc in range(4):
                groups.append(qk_group("q", oc))
            for oc in range(4):
                groups.append(qk_group("k", oc))
            for lsub in range(4):
                groups.append(v_group(lsub))
            return qt_ic, groups

        def emit_B(ic, qt_ic, qres, agroups):
            """Attention for query chunk ic; A(ic+1) groups interleaved."""
            # jc blocks. For ic>0: off-diagonal first (full width, no
            # affine_selects on the head's critical path; the first AV
            # covers all columns for the PSUM start); diagonals last.
            # For ic==0 there are only diagonals; rr0 leads (full width).
            jcs = []
            for jc in range(4 * ic):
                jcs.append((jc, 512, 0, False))
            for rr in range(4):
                jcs.append((4 * ic + rr, 512 - rr * 128, rr * 128, True))
            prs = [(jcs[2 * i], jcs[2 * i + 1]) for i in range(len(jcs) // 2)]
            npr = len(prs)
            a_done = 0

            for h8 in range(H8):
                oc = h8 // 2
                po = (h8 % 2) * 64
                ot = otpp.tile([80, 512], F32, tag="ot", name="ot")

                def st_exp(pi):
                    (jc0, w0, i0a, d0), (jc1, w1, i0b, d1) = prs[pi]
                    st = stpp.tile([128, 1024], F32, tag="st", name="st")
                    nc.tensor.matmul(
                        st[:, 0:w0],
                        kt_sb[po:po + 64,
                              oc * l + jc0 * 128:oc * l + jc0 * 128 + 128],
                        qt_ic[po:po + 64,
                              oc * 512 + i0a:oc * 512 + i0a + w0],
                        start=True, stop=True)
                    nc.tensor.matmul(
                        st[:, w0:w0 + w1],
                        kt_sb[po:po + 64,
                              oc * l + jc1 * 128:oc * l + jc1 * 128 + 128],
                        qt_ic[po:po + 64,
                              oc * 512 + i0b:oc * 512 + i0b + w1],
                        start=True, stop=True)
                    at = atp.tile([128, 1024], F8, tag="at", name="at")
                    nc.scalar.activation(
                        at[:, 0:w0 + w1], st[:, 0:w0 + w1],
                        mybir.ActivationFunctionType.Exp, scale=SCALE)
                    # partial diagonal 128-block: keep j <= i
                    for (dflag, c0) in ((d0, 0), (d1, w0)):
                        if dflag:
                            nc.gpsimd.affine_select(
                                out=at[:, c0:c0 + 128],
                                in_=at[:, c0:c0 + 128],
                                compare_op=ALU.is_ge,
                                fill=0.0,
                                base=0,
                                pattern=[[1, 128]],
                                channel_multiplier=-1,
                            )
                    return at

                def av(pi, at):
                    (jc0, w0, i0a, d0), (jc1, w1, i0b, d1) = prs[pi]
                    if not d0 and not d1:
                        # both off-diagonal, full width: one DoubleRow
                        # matmul sums both jc blocks' contributions
                        vpair = v_sb[:, (jc0 // 2) * 1280 + h8 * 160:
                                     (jc0 // 2) * 1280 + h8 * 160 + 160]
                        nc.tensor.matmul(
                            ot,
                            vpair.rearrange("p (two c) -> p two c", two=2),
                            at.rearrange("p (two c) -> p two c", two=2),
                            start=(pi == 0),
                            stop=False,
                            perf_mode=DR,
                            skip_group_check=True)
                    else:
                        for (jc, w, i0, c0) in ((jc0, w0, i0a, 0),
                                                (jc1, w1, i0b, w0)):
                            voff = ((jc // 2) * 1280 + h8 * 160
                                    + (jc % 2) * 80)
                            nc.tensor.matmul(
                                ot[0:65, i0:i0 + w],
                                v_sb[:, voff:voff + 65],
                                at[:, c0:c0 + w],
                                start=(pi == 0 and c0 == 0),
                                stop=(pi == npr - 1 and c0 == w0),
                                skip_group_check=True)

                at_prev = st_exp(0)
                for pi in range(1, npr):
                    at_cur = st_exp(pi)
                    av(pi - 1, at_prev)
                    at_prev = at_cur
                av(npr - 1, at_prev)

                # normalize (softmax divide) + write res^T slice
                rr_t = rrp.tile([1, 512], F32, tag="rr", name="rr")
                nc.vector.reciprocal(rr_t, ot[64:65, :])
                rb = rbp.tile([64, 512], F32, tag="rb", name="rb")
                nc.gpsimd.partition_broadcast(rb, rr_t)
                resv_h = res_sb[po:po + 64, oc * l + ic * 512:
                                oc * l + ic * 512 + 512]
                nc.vector.tensor_tensor(resv_h, ot[0:64, :], rb,
                                        op=ALU.mult)

                if h8 % 2 == 1:
                    # both heads of dtile oc done: residual + BN stats
                    resv = res_sb[:, oc * l + ic * 512:
                                  oc * l + ic * 512 + 512]
                    nc.vector.tensor_add(resv, resv,
                                         qres[:, oc * 512:(oc + 1) * 512])
                    acc = accs[0]
                    sq_t = sqp.tile([128, 512], F32, tag="sq", name="sq")
                    nc.gpsimd.tensor_mul(sq_t, resv, resv)
                    if ic == 0:
                        nc.vector.reduce_sum(acc[:, oc:oc + 1], resv,
                                             axis=AX)
                        nc.vector.reduce_sum(acc[:, 4 + oc:5 + oc], sq_t,
                                             axis=AX)
                    else:
                        r1 = redp.tile([128, 1], F32, tag="red", name="red")
                        nc.vector.reduce_sum(r1, resv, axis=AX)
                        nc.vector.tensor_add(acc[:, oc:oc + 1],
                                             acc[:, oc:oc + 1], r1)
                        r2 = redp.tile([128, 1], F32, tag="red", name="red")
                        nc.vector.reduce_sum(r2, sq_t, axis=AX)
                        nc.gpsimd.tensor_add(acc[:, 4 + oc:5 + oc],
                                             acc[:, 4 + oc:5 + oc], r2)

                # interleave next chunk's projection groups
                want = (12 * (h8 + 1)) // H8
                while a_done < min(want, len(agroups)):
                    agroups[a_done]()
                    a_done += 1
            while a_done < len(agroups):
                agroups[a_done]()
                a_done += 1

        # ---------------- main pipeline --------------------------------
        qt_cur, g0 = emit_A_groups(0, qxt0, kxt0, on_act=True)
        for g in g0:
            g()
        qres_cur = qres0
        for ic in range(ic_n):
            if ic + 1 < ic_n:
                qxt_next = dma_xt(ic + 1, "q")
                kxt_next = dma_xt(ic + 1, "k")
                qres_next = dma_qres(ic + 1)
                qt_next, agroups = emit_A_groups(
                    ic + 1, qxt_next, kxt_next, on_act=(ic == 0))
            else:
                agroups = []
            emit_B(ic, qt_cur, qres_cur, agroups)
            if ic + 1 < ic_n:
                qt_cur, qres_cur = qt_next, qres_next

        # ---------------- collective + BN ------------------------------
        cc_in = dramp.tile([1, 2 * F], F32, tag="cc_in")
        cc_out = dramp.tile([4, 2 * F], F32, tag="cc_out")

        gamma_sb = consts.tile([128, 4], F32)
        nc.scalar.dma_start(gamma_sb, bass.AP(
            tensor=gamma.tensor, offset=gamma.offset, ap=[[1, 128], [128, 4]]))
        beta_sb = consts.tile([128, 4], F32)
        nc.scalar.dma_start(beta_sb, bass.AP(
            tensor=beta.tensor, offset=beta.offset, ap=[[1, 128], [128, 4]]))

        nc.sync.dma_start(
            bass.AP(tensor=cc_in.tensor, offset=cc_in.offset,
                    ap=[[1, 128], [128, 8]]),
            accs[0])

        nc.gpsimd.collective_compute(
            "AllGather",
            ALU.bypass,
            replica_groups=[[0, 1, 2, 3], [4, 5, 6, 7]],
            ins=[cc_in],
            outs=[cc_out],
        )

        g8 = bnp.tile([128, 8, 4], F32, tag="g8", name="g8")
        rengs = (nc.sync, nc.scalar, nc.gpsimd, nc.sync)
        for rank in range(4):
            rengs[rank].dma_start(g8[:, :, rank], bass.AP(
                tensor=cc_out.tensor,
                offset=cc_out.offset + rank * 2 * F,
                ap=[[1, 128], [128, 8]]))
        gsum = bnp.tile([128, 8], F32, tag="gsum", name="gsum")
        nc.vector.reduce_sum(gsum, g8, axis=AX)
        mom = bnp.tile([128, 8], F32, tag="mom", name="mom")
        nc.vector.tensor_scalar_mul(mom, gsum, 1.0 / nrows_total)
        m2 = bnp.tile([128, 4], F32, tag="m2", name="m2")
        nc.vector.tensor_mul(m2, mom[:, 0:4], mom[:, 0:4])
        var = bnp.tile([128, 4], F32, tag="var", name="var")
        nc.vector.tensor_sub(var, mom[:, 4:8], m2)
        # rstd = 1/sqrt(var+eps) via bit-trick + 2 Newton steps (DVE only;
        # avoids the 1.3us activation-table reloads at the tail)
        I32 = mybir.dt.int32
        xe = bnp.tile([128, 4], F32, tag="xe", name="xe")
        nc.vector.tensor_scalar_add(xe, var, EPS)
        yi = bnp.tile([128, 4], F32, tag="yi", name="yi")
        nc.vector.tensor_scalar(yi.bitcast(I32), xe.bitcast(I32),
                                1, None, op0=ALU.logical_shift_right)
        nc.vector.tensor_scalar(yi.bitcast(I32), yi.bitcast(I32),
                                -1, 0x5F3759DF, op0=ALU.mult, op1=ALU.add)
        rstd = yi
        for _ in range(2):
            t = bnp.tile([128, 4], F32, tag=f"nt{_}", name="nt")
            nc.vector.tensor_mul(t, xe, rstd)
            nc.vector.tensor_mul(t, t, rstd)
            nc.vector.tensor_scalar(t, t, -0.5, 1.5,
                                    op0=ALU.mult, op1=ALU.add)
            y2 = bnp.tile([128, 4], F32, tag=f"ny{_}", name="ny")
            nc.vector.tensor_mul(y2, rstd, t)
            rstd = y2
        gp = bnp.tile([128, 4], F32, tag="gp", name="gp")
        nc.vector.tensor_mul(gp, gamma_sb, rstd)
        mgp = bnp.tile([128, 4], F32, tag="mgp", name="mgp")
        nc.vector.tensor_mul(mgp, mom[:, 0:4], gp)
        bp = bnp.tile([128, 4], F32, tag="bp", name="bp")
        nc.vector.tensor_sub(bp, beta_sb, mgp)

        # apply: out = res * gp + bp, per-partition scalars; one tile per
        # dtile, alternating DVE/Pool, out-DMAs spread across queues
        dengs = (nc.sync, nc.scalar, nc.gpsimd, nc.sync,
                 nc.scalar, nc.gpsimd, nc.sync, nc.scalar)
        for k in range(8):
            dc, half = k // 2, k % 2
            src = res_sb[:, dc * l + half * 1024:dc * l + half * 1024 + 1024]
            o_t = outp.tile([128, 1024], BF16, tag="o", name="o")
            nc.vector.tensor_scalar(o_t, src, gp[:, dc:dc + 1],
                                    bp[:, dc:dc + 1],
                                    op0=ALU.mult, op1=ALU.add)
            dengs[k].dma_start(
                out_s[dc * 128:(dc + 1) * 128,
                      half * 1024:half * 1024 + 1024], o_t)

    nc.compile()
    return nc


def get_runner(nc):
    """Build (once) a cached jitted SPMD executor for the Bass program."""
    if "runner" in _cached:
        return _cached["runner"]

    import jax
    from jax.experimental.shard_map import shard_map
    from jax.sharding import Mesh, PartitionSpec
    from concourse import bass2jax

    bass2jax.install_neuronx_cc_hook()

    partition_name = (nc.partition_id_tensor.name
                      if nc.partition_id_tensor else None)
    in_names, out_names, out_avals, zero_outs = [], [], [], []
    for alloc in nc.m.functions[0].allocations:
        if not isinstance(alloc, mybir.MemoryLocationSet):
            continue
        name = alloc.memorylocations[0].name
        if alloc.kind == "ExternalInput":
            if name != partition_name:
                in_names.append(name)
        elif alloc.kind == "ExternalOutput":
            shape = tuple(alloc.tensor_shape)
            dtype = mybir.dt.np(alloc.dtype)
            out_names.append(name)
            out_avals.append(jax.core.ShapedArray(shape, dtype))
            zero_outs.append(np.zeros(shape, dtype))
    n_params = len(in_names)
    n_outs = len(out_avals)
    all_names = in_names + out_names
    if partition_name is not None:
        all_names = all_names + [partition_name]

    def _body(*args):
        operands = list(args)
        if partition_name is not None:
            operands.append(bass2jax.partition_id_tensor())
        outs = bass2jax._bass_exec_p.bind(
            *operands,
            out_avals=tuple(out_avals),
            in_names=tuple(all_names),
            out_names=tuple(out_names),
            lowering_input_output_aliases=(),
            sim_require_finite=True,
            sim_require_nnan=True,
            nc=nc,
        )
        return tuple(outs)

    devices = jax.devices()[:NCORES]
    mesh = Mesh(np.asarray(devices), ("core",))
    in_specs = (PartitionSpec("core"),) * (n_params + n_outs)
    out_specs = (PartitionSpec("core"),) * n_outs
    donate = tuple(range(n_params, n_params + n_outs))
    sharded = jax.jit(
        shard_map(_body, mesh=mesh, in_specs=in_specs, out_specs=out_specs,
                  check_rep=False),
        donate_argnums=donate, keep_unused=True)

    def run_np(in_maps):
        concat_in = [
            np.concatenate([np.asarray(in_maps[c][nm]) for c in range(NCORES)],
                           axis=0)
            for nm in in_names]
        concat_zeros = [np.zeros((NCORES * z.shape[0], *z.shape[1:]), z.dtype)
                        for z in zero_outs]
        out_arrs = sharded(*concat_in, *concat_zeros)
        return [
            {nm: np.asarray(out_arrs[i]).reshape(
                NCORES, *out_avals[i].shape)[c]
             for i, nm in enumerate(out_names)}
            for c in range(NCORES)]

    _cached["runner"] = (run_np, sharded, in_names, out_names, out_avals,
                         zero_outs, mesh)
    return _cached["runner"]


def make_in_maps(inputs, l):
    import ml_dtypes
    bf16 = ml_dtypes.bfloat16
    fp8 = mybir.dt.np(F8)

    query = np.asarray(inputs["query"], dtype=np.float32)
    key = np.asarray(inputs["key"], dtype=np.float32)
    Wq = np.asarray(inputs["Wq"], dtype=np.float32)
    Wk = np.asarray(inputs["Wk"], dtype=np.float32)
    Wv = np.asarray(inputs["Wv"], dtype=np.float32)
    gamma = np.asarray(inputs["gamma"], dtype=np.float32)
    beta = np.asarray(inputs["beta"], dtype=np.float32)

    in_maps = []
    for c in range(NCORES):
        n, hb = c % NB, c // NB
        sl = slice(hb * F, (hb + 1) * F)
        # permute the contraction dim: this core's own residual dims
        # (= its output feature slice) first, the rest after; W rows
        # are permuted identically, so projections are unchanged.
        perm = np.r_[hb * F:(hb + 1) * F,
                     0:hb * F, (hb + 1) * F:D]
        qT = query[n].T

        def wtile_qk(W):
            # [p, dp*1024 + oc*256 + t*128 + c] = W^T[(2dp+t)*128+p, oc*128+c]
            WT = W[sl].T[perm]
            return np.ascontiguousarray(
                WT.reshape(4, 2, 128, 4, 128).transpose(2, 0, 3, 1, 4)
                .reshape(128, 4096).astype(fp8))

        def wtile_v(W):
            # dc-major: [p, dc*512 + f] = W^T[dc*128+p, f]
            WT = W[sl].T[perm]
            return np.ascontiguousarray(
                WT.reshape(8, 128, 512).transpose(1, 0, 2)
                .reshape(128, 4096).astype(fp8))

        in_maps.append({
            "qt_nd": np.ascontiguousarray(qT[perm].astype(fp8)),
            "kt_nd": np.ascontiguousarray(key[n].T[perm].astype(fp8)),
            "qres_nd": np.ascontiguousarray(qT[sl].astype(bf16)),
            "wqt": wtile_qk(Wq),
            "wkt": wtile_qk(Wk),
            "wvt": wtile_v(Wv),
            "gamma": np.ascontiguousarray(gamma[sl].reshape(1, F)),
            "beta": np.ascontiguousarray(beta[sl].reshape(1, F)),
        })
    return in_maps


def kernel(**inputs):
    l = np.asarray(inputs["query"]).shape[1]
    if "nc" not in _cached or _cached.get("l") != l:
        _cached["nc"] = build_program(l)
        _cached["l"] = l
    nc = _cached["nc"]

    in_maps = make_in_maps(inputs, l)
    run_np = get_runner(nc)[0]
    results = run_np(in_maps)

    out = np.zeros((N, l, D), dtype=np.float32)
    for c in range(NCORES):
        n, hb = c % NB, c // NB
        out[n, :, hb * F:(hb + 1) * F] = \
            results[c]["out_s"].T.astype(np.float32)
    return out
